# revision 1
# baseline (speedup 1.0000x reference)
"""ConvLRUBlock Trainium2 kernel.

Reference computation (per batch b):
    h   = rms_norm(x, norm_w)                  # over channel dim
    uv  = conv3d_3x3x3(h, w_in) + b_in         # pad: replicate T/H, circular W
    u   = silu(a) * g          (a, g = uv split on channels)
    y_t = Re(h_t) c_re + Im(h_t) c_im,  h_t = lam h_{t-1} + gamma u_t  (diag LRU)
    out = x + conv3d_3x3x3(y, w_out) + b_out

Sharding: 8 cores = (batch 2) x (H quarters 4). Each core receives its H
slice plus 2 halo rows each side (edge-replicated) and the W dim circularly
padded to W+2, so no inter-core communication is needed. All conv padding is
resolved by host-side halo materialization + in-kernel index clamping (T) +
in-SBUF wrap-column fixes (W for the second conv).

In-kernel layout: channels (96) on SBUF partitions; spatial (rows x (W+2))
flattened on the free dim. 3x3x3 convs = 27 accumulating matmuls per output
tile; kh/kw become column shifts of the rhs AP, kt picks one of 3 t-slabs.
The LRU scan is 16 sequential complex steps on the vector engine.
"""

import os
from contextlib import ExitStack

import ml_dtypes
import numpy as np

import concourse.bacc as bacc
import concourse.bass as bass  # noqa: F401
import concourse.tile as tile
from concourse import mybir

F32 = mybir.dt.float32
BF16 = mybir.dt.bfloat16
ALU = mybir.AluOpType
AF = mybir.ActivationFunctionType

EPS = 1e-6

# Full-problem constants
B_FULL, C_FULL, T_FULL, H_FULL, W_FULL = 2, 96, 16, 64, 128
QH = 4  # H quarters
N_CORES = 8


def build_program(C=96, T=16, HR=16, W=128, CT=512, use_silu=True,
                  pack=False, pack2=False):
    """Build the single-core SPMD Bass program.

    C: channels; T: time steps; HR: output H rows per core; W: width.
    CT: matmul/psum column tile (<=512). use_silu: Silu on ACT vs
    Sigmoid+mults (the simulator does not implement Silu).
    """
    Wp = W + 2           # circular-padded width
    RIN = HR + 4         # input rows (2 halo each side, for two convs)
    RU = HR + 2          # u/y rows (1 halo each side, for conv_out)
    NIN = RIN * Wp       # flattened input cols per t
    NU = RU * Wp         # flattened u/y cols per t
    NO = HR * Wp         # flattened output cols per t

    nc = bacc.Bacc()
    xh = nc.declare_dram_parameter("xh", [C, T, RIN, Wp], F32, isOutput=False)
    w_in = nc.declare_dram_parameter("w_in", [C, 27, 2 * C], BF16, isOutput=False)
    w_out = nc.declare_dram_parameter("w_out", [C, 27, C], BF16, isOutput=False)
    onesw = nc.declare_dram_parameter("onesw", [C, 128], BF16, isOutput=False)
    consts = nc.declare_dram_parameter("consts", [C, 13], F32, isOutput=False)
    consts2 = nc.declare_dram_parameter("consts2", [128, 2], F32, isOutput=False)
    out = nc.declare_dram_parameter("out", [C, T, HR, W], F32, isOutput=True)

    def col_tiles(total):
        return [(i, min(CT, total - i)) for i in range(0, total, CT)]

    with tile.TileContext(nc) as tc, ExitStack() as ctx:
        singles = ctx.enter_context(tc.tile_pool(name="singles", bufs=1))
        xpool = ctx.enter_context(tc.tile_pool(name="xpool", bufs=2))
        sqpool = ctx.enter_context(tc.tile_pool(name="sqpool", bufs=2))
        statpool = ctx.enter_context(tc.tile_pool(name="statpool", bufs=2))
        hnpool = ctx.enter_context(tc.tile_pool(name="hnpool", bufs=4))
        sapool = ctx.enter_context(tc.tile_pool(name="sapool", bufs=3))
        bpool = ctx.enter_context(tc.tile_pool(name="bpool", bufs=2))
        hrpool = ctx.enter_context(tc.tile_pool(name="hrpool", bufs=2))
        hipool = ctx.enter_context(tc.tile_pool(name="hipool", bufs=2))
        tmppool = ctx.enter_context(tc.tile_pool(name="tmppool", bufs=2))
        ypool = ctx.enter_context(tc.tile_pool(name="ypool", bufs=4))
        opool = ctx.enter_context(tc.tile_pool(name="opool", bufs=2))
        psN = ctx.enter_context(tc.tile_pool(name="psN", bufs=2, space="PSUM"))
        psA = ctx.enter_context(tc.tile_pool(name="psA", bufs=2, space="PSUM"))
        psG = ctx.enter_context(tc.tile_pool(name="psG", bufs=2, space="PSUM"))
        psO = ctx.enter_context(tc.tile_pool(name="psO", bufs=2, space="PSUM"))

        sb_win = singles.tile([C, 27, 2 * C], BF16)
        nc.sync.dma_start(out=sb_win[:], in_=w_in[:])
        sb_wout = singles.tile([C, 27, C], BF16)
        nc.sync.dma_start(out=sb_wout[:], in_=w_out[:])
        sb_ones = singles.tile([C, 128], BF16)
        nc.sync.dma_start(out=sb_ones[:], in_=onesw[:])
        sb_c = singles.tile([C, 13], F32)
        nc.sync.dma_start(out=sb_c[:], in_=consts[:])
        sb_c2 = singles.tile([128, 2], F32)
        nc.sync.dma_start(out=sb_c2[:], in_=consts2[:])
        c_aux0 = sb_c2[:, 0:1]
        c_aux1 = sb_c2[:, 1:2]
        c_ba = sb_c[:, 0:1]
        c_bg = sb_c[:, 1:2]
        c_lr = sb_c[:, 2:3]
        c_li = sb_c[:, 3:4]
        c_nli = sb_c[:, 4:5]
        c_gcre = sb_c[:, 5:6]
        c_gcim = sb_c[:, 6:7]
        c_bout = sb_c[:, 7:8]
        c_m0 = sb_c[:, 8:9]
        c_1m0 = sb_c[:, 9:10]
        c_m1 = sb_c[:, 10:11]
        c_1m1 = sb_c[:, 11:12]
        c_eps = sb_c[:, 12:13]

        # Warm-up reads: make each compute engine observe the const-DMA
        # semaphores early, so steady-state ops carry at most one sync wait
        # (walrus rejects DVE ops with two wait commands).
        wu_v = singles.tile([C, 13], F32)
        nc.vector.tensor_copy(wu_v[:], sb_c[:])
        wu_s = singles.tile([C, 13], F32)
        nc.scalar.activation(wu_s[:], sb_c[:], AF.Square)

        touchpool = ctx.enter_context(tc.tile_pool(name="touchpool", bufs=2))
        if pack:
            gspool = ctx.enter_context(tc.tile_pool(name="gspool", bufs=2))
            gfpool = ctx.enter_context(tc.tile_pool(name="gfpool", bufs=2))
        if pack2:
            piecepool = ctx.enter_context(tc.tile_pool(name="piecepool", bufs=2))

        def touch(ap, engines="v"):
            """Tiny read of a freshly-DMA'd tile so the engine observes the
            DMA-queue semaphore here; later big consumers then carry only
            engine-sem waits (walrus rejects DVE ops with 2 sync waits)."""
            if "v" in engines:
                tv = touchpool.tile([C, 1], F32, tag="tv")
                nc.vector.tensor_copy(tv[:], ap)
            if "s" in engines:
                ts_ = touchpool.tile([C, 1], F32, tag="ts")
                nc.scalar.activation(ts_[:], ap, AF.Square)

        hn_slabs = [None] * T  # hnorm tiles, data at col offset 1
        y_slabs = [None] * T   # y tiles (bf16), data at col offset 1

        def stage_a(t):
            """x[t] -> hnorm[t] (rms-normed, bf16, [C, 1+NIN+1])."""
            xt = xpool.tile([C, RIN, Wp], F32, tag="xt")
            nc.sync.dma_start(out=xt[:], in_=xh[:, t])
            touch(xt[:, 0, 0:1], engines="vs")
            xf = xt[:].rearrange("p r w -> p (r w)")
            hn = hnpool.tile([C, 1 + NIN + 1], BF16, tag="hn")
            nc.vector.memset(hn[:, 0:1], 0.0)
            nc.vector.memset(hn[:, 1 + NIN:], 0.0)
            for c0, n in col_tiles(NIN):
                sq = sqpool.tile([C, CT], BF16, tag="sq")
                nc.scalar.activation(sq[:, :n], xf[:, c0:c0 + n], AF.Square)
                ps = psN.tile([128, CT], F32, tag="psn")
                nc.tensor.matmul(ps[:, :n], sb_ones[:], sq[:, :n],
                                 start=True, stop=True)
                lg = statpool.tile([C, CT], F32, tag="lg")
                nc.scalar.activation(lg[:, :n], ps[:C, :n], AF.Ln,
                                     scale=1.0 / C, bias=c_eps)
                inv = statpool.tile([C, CT], F32, tag="inv")
                nc.scalar.activation(inv[:, :n], lg[:, :n], AF.Exp, scale=-0.5)
                nc.vector.tensor_mul(hn[:, 1 + c0:1 + c0 + n],
                                     xf[:, c0:c0 + n], inv[:, :n])
            hn_slabs[t] = hn
            return hn

        def gate_epilogue(pa, pg_sb, bt, c0, n):
            """silu(a+ba)*(g+bg) for one coltile; a=pa[0:C] (psum),
            g already realigned to pg_sb [C, n] (sbuf)."""
            if use_silu:
                sa = sapool.tile([C, CT], BF16, tag="sa")
                nc.scalar.activation(sa[:, :n], pa[:C, :n], AF.Silu,
                                     bias=c_ba)
            else:
                sg = sapool.tile([C, CT], BF16, tag="sg")
                nc.scalar.activation(sg[:, :n], pa[:C, :n], AF.Sigmoid,
                                     bias=c_ba)
                av = sapool.tile([C, CT], F32, tag="av")
                nc.vector.scalar_tensor_tensor(av[:, :n], pa[:C, :n], c_ba,
                                               sg[:, :n], ALU.add,
                                               ALU.bypass)
                sa = sapool.tile([C, CT], BF16, tag="sa")
                nc.vector.tensor_mul(sa[:, :n], sg[:, :n], av[:, :n])
            nc.vector.scalar_tensor_tensor(bt[:, c0:c0 + n], pg_sb[:, :n],
                                           c_bg, sa[:, :n],
                                           ALU.add, ALU.mult)

        def conv_in_packed(t):
            """1.5-array-pass conv_in: pass1 M=128 (a0..95,g0..31), pass2
            col-tiled pairs of M=64 (g32..95) for two coltiles at once."""
            slabs = [hn_slabs[min(max(t + kt - 1, 0), T - 1)] for kt in range(3)]
            bt = bpool.tile([C, NU], BF16, tag="bt")
            cts = col_tiles(NU)
            for p0 in range(0, len(cts), 2):
                pair = cts[p0:p0 + 2]
                pas = []
                for c0, n in pair:
                    pa = psA.tile([128, CT], F32, tag="pa")
                    for kt in range(3):
                        rhs_all = slabs[kt]
                        for kh in range(3):
                            for kw in range(3):
                                off = kt * 9 + kh * 3 + kw
                                s = 1 + c0 + kh * Wp + kw - 1
                                nc.tensor.matmul(
                                    pa[:, :n], sb_win[:, off, 0:128],
                                    rhs_all[:, s:s + n],
                                    start=(off == 0), stop=(off == 26))
                    pas.append(pa)
                pg = psG.tile([128, CT], F32, tag="pg")
                for kt in range(3):
                    rhs_all = slabs[kt]
                    for kh in range(3):
                        for kw in range(3):
                            off = kt * 9 + kh * 3 + kw
                            for j, (c0, n) in enumerate(pair):
                                s = 1 + c0 + kh * Wp + kw - 1
                                b = 64 * j
                                nc.tensor.matmul(
                                    pg[b:b + 64, :n],
                                    sb_win[:, off, 128:192],
                                    rhs_all[:, s:s + n],
                                    start=(off == 0), stop=(off == 26),
                                    tile_position=(0, b),
                                    skip_group_check=True)
                for j, (c0, n) in enumerate(pair):
                    b = 64 * j
                    pa = pas[j]
                    gsa = gspool.tile([128, CT], F32, tag="gsa")
                    nc.vector.tensor_copy(gsa[96:128, :n], pa[96:128, :n])
                    gsb = gspool.tile([128, CT], F32, tag="gsb")
                    nc.vector.tensor_copy(gsb[b:b + 64, :n], pg[b:b + 64, :n])
                    gf = gfpool.tile([C, CT], F32, tag="gf")
                    nc.sync.dma_start(out=gf[0:32, :n], in_=gsa[96:128, :n])
                    nc.sync.dma_start(out=gf[32:96, :n], in_=gsb[b:b + 64, :n])
                    gate_epilogue(pa, gf, bt, c0, n)
            return bt

        def conv_in(t):
            """hnorm[t-1..t+1] -> b[t] = silu(a+ba)*(g+bg), bf16 [C, NU]."""
            slabs = [hn_slabs[min(max(t + kt - 1, 0), T - 1)] for kt in range(3)]
            bt = bpool.tile([C, NU], BF16, tag="bt")
            for c0, n in col_tiles(NU):
                pa = psA.tile([C, CT], F32, tag="pa")
                pg = psG.tile([C, CT], F32, tag="pg")
                for kt in range(3):
                    rhs_all = slabs[kt]
                    for kh in range(3):
                        for kw in range(3):
                            off = kt * 9 + kh * 3 + kw
                            s = 1 + c0 + kh * Wp + kw - 1
                            rhs = rhs_all[:, s:s + n]
                            nc.tensor.matmul(pa[:, :n], sb_win[:, off, 0:C],
                                             rhs, start=(off == 0),
                                             stop=(off == 26))
                for kt in range(3):
                    rhs_all = slabs[kt]
                    for kh in range(3):
                        for kw in range(3):
                            off = kt * 9 + kh * 3 + kw
                            s = 1 + c0 + kh * Wp + kw - 1
                            rhs = rhs_all[:, s:s + n]
                            nc.tensor.matmul(pg[:, :n], sb_win[:, off, C:2 * C],
                                             rhs, start=(off == 0),
                                             stop=(off == 26))
                gate_epilogue(pa, pg, bt, c0, n)
            return bt

        scan_state = [None, None]  # hr, hi tiles [C, NU] f32

        def scan_step(t, bt):
            """LRU step + projection -> y[t] (bf16 slab, data at offset 1)."""
            hr_new = hrpool.tile([C, NU], F32, tag="hr")
            hi_new = hipool.tile([C, NU], F32, tag="hi")
            if t == 0:
                nc.vector.tensor_copy(hr_new[:], bt[:])
                nc.vector.memset(hi_new[:], 0.0)
            else:
                hr_old, hi_old = scan_state
                t1 = tmppool.tile([C, NU], F32, tag="tA")
                nc.vector.scalar_tensor_tensor(t1[:], hi_old[:], c_nli, bt[:],
                                               ALU.mult, ALU.add)
                nc.vector.scalar_tensor_tensor(hr_new[:], hr_old[:], c_lr,
                                               t1[:], ALU.mult, ALU.add)
                t2 = tmppool.tile([C, NU], F32, tag="tB")
                nc.vector.scalar_tensor_tensor(t2[:], hi_old[:], c_lr,
                                               hi_old[:], ALU.mult, ALU.bypass)
                nc.vector.scalar_tensor_tensor(hi_new[:], hr_old[:], c_li,
                                               t2[:], ALU.mult, ALU.add)
            scan_state[0], scan_state[1] = hr_new, hi_new
            t3 = tmppool.tile([C, NU], F32, tag="tA")
            nc.vector.scalar_tensor_tensor(t3[:], hr_new[:], c_gcre,
                                           hr_new[:], ALU.mult, ALU.bypass)
            yt = ypool.tile([C, 1 + NU + 1], BF16, tag="yt")
            nc.vector.memset(yt[:, 0:1], 0.0)
            nc.vector.memset(yt[:, 1 + NU:], 0.0)
            nc.vector.scalar_tensor_tensor(yt[:, 1:1 + NU], hi_new[:], c_gcim,
                                           t3[:], ALU.mult, ALU.add)
            # W wrap columns: col 0 <- col W (w=W-1), col W+1 <- col 1 (w=0)
            yv = yt[:, 1:1 + NU].rearrange("p (r w) -> p r w", w=Wp)
            nc.vector.tensor_copy(yv[:, :, 0:1], yv[:, :, W:W + 1])
            nc.vector.tensor_copy(yv[:, :, W + 1:W + 2], yv[:, :, 1:2])
            # H edge replication (active only on global-edge cores, via mask):
            # row0 <- m0*row0 + (1-m0)*row1 ; last <- m1*last + (1-m1)*prev
            e0 = tmppool.tile([C, Wp], F32, tag="tE")
            nc.vector.scalar_tensor_tensor(e0[:], yv[:, 1, :], c_1m0,
                                           yv[:, 1, :], ALU.mult, ALU.bypass)
            nc.vector.scalar_tensor_tensor(yv[:, 0, :], yv[:, 0, :], c_m0,
                                           e0[:], ALU.mult, ALU.add)
            e1 = tmppool.tile([C, Wp], F32, tag="tE")
            nc.vector.scalar_tensor_tensor(e1[:], yv[:, RU - 2, :], c_1m1,
                                           yv[:, RU - 2, :], ALU.mult, ALU.bypass)
            nc.vector.scalar_tensor_tensor(yv[:, RU - 1, :], yv[:, RU - 1, :],
                                           c_m1, e1[:], ALU.mult, ALU.add)
            y_slabs[t] = yt
            return yt

        def conv_out(t):
            """y[t-1..t+1] -> out[t] = x + conv(y) + b_out."""
            slabs = [y_slabs[min(max(t + kt - 1, 0), T - 1)] for kt in range(3)]
            ot = opool.tile([C, HR, Wp], F32, tag="ot")
            # residual input loaded into the output staging tile
            nc.sync.dma_start(out=ot[:], in_=xh[:, t, 2:2 + HR, :])
            touch(ot[:, 0, 0:1], engines="v")
            of = ot[:].rearrange("p r w -> p (r w)")
            for c0, n in col_tiles(NO):
                po = psO.tile([C, CT], F32, tag="po")
                for kt in range(3):
                    rhs_all = slabs[kt]
                    for kh in range(3):
                        for kw in range(3):
                            off = kt * 9 + kh * 3 + kw
                            s = 1 + c0 + kh * Wp + kw - 1
                            rhs = rhs_all[:, s:s + n]
                            nc.tensor.matmul(po[:, :n], sb_wout[:, off, :],
                                             rhs, start=(off == 0),
                                             stop=(off == 26))
                nc.vector.scalar_tensor_tensor(of[:, c0:c0 + n], po[:, :n],
                                               c_bout, of[:, c0:c0 + n],
                                               ALU.add, ALU.add)
            nc.sync.dma_start(out=out[:, t], in_=ot[:, :, 1:1 + W])

        def conv_out_packed(t):
            """conv_out with array packing: pairs (e,o) put e's 96 channels +
            o's first 32 (pos 96) in one pass; the two pairs' leftover 64
            channels share one concurrent col-tiled pass. Misplaced pieces are
            realigned into the staging tile via SBUF->SBUF DMA."""
            slabs = [y_slabs[min(max(t + kt - 1, 0), T - 1)] for kt in range(3)]
            ot = opool.tile([C, HR, Wp], F32, tag="ot")
            nc.sync.dma_start(out=ot[:], in_=xh[:, t, 2:2 + HR, :])
            touch(ot[:, 0, 0:1], engines="v")
            of = ot[:].rearrange("p r w -> p (r w)")
            xflat = xh[:, t, 2:2 + HR, :].rearrange("p r w -> p (r w)")
            cts = col_tiles(NO)

            def mm_group(ps, prange, wslice, c0, n, pos):
                for kt in range(3):
                    rhs_all = slabs[kt]
                    for kh in range(3):
                        for kw in range(3):
                            off = kt * 9 + kh * 3 + kw
                            s = 1 + c0 + kh * Wp + kw - 1
                            nc.tensor.matmul(
                                ps[prange[0]:prange[1], :n],
                                sb_wout[:, off, wslice[0]:wslice[1]],
                                rhs_all[:, s:s + n],
                                start=(off == 0), stop=(off == 26),
                                tile_position=pos, skip_group_check=True)

            def mm_group2(ps, jobs):
                """Interleaved concurrent accumulation groups."""
                for kt in range(3):
                    rhs_all = slabs[kt]
                    for kh in range(3):
                        for kw in range(3):
                            off = kt * 9 + kh * 3 + kw
                            for prange, wslice, c0, n, pos in jobs:
                                s = 1 + c0 + kh * Wp + kw - 1
                                nc.tensor.matmul(
                                    ps[prange[0]:prange[1], :n],
                                    sb_wout[:, off, wslice[0]:wslice[1]],
                                    rhs_all[:, s:s + n],
                                    start=(off == 0), stop=(off == 26),
                                    tile_position=pos, skip_group_check=True)

            def aligned_epi(ps, c0, n):
                nc.vector.scalar_tensor_tensor(of[:, c0:c0 + n], ps[:C, :n],
                                               c_bout, of[:, c0:c0 + n],
                                               ALU.add, ALU.add)

            def piece_epi(ps, pbase, psize, ch0, c0, n):
                """Residual-add for a channel piece at partitions
                [pbase, pbase+psize) holding channels [ch0, ch0+psize);
                realign into ot via DMA."""
                xp = piecepool.tile([128, CT], F32, tag="xp")
                nc.sync.dma_start(out=xp[pbase:pbase + psize, :n],
                                  in_=xflat[ch0:ch0 + psize, c0:c0 + n])
                aux = c_aux0 if pbase == 96 else c_aux1
                pt = piecepool.tile([128, CT], F32, tag="pc")
                nc.vector.scalar_tensor_tensor(
                    pt[pbase:pbase + psize, :n],
                    ps[pbase:pbase + psize, :n],
                    aux[pbase:pbase + psize, :],
                    xp[pbase:pbase + psize, :n], ALU.add, ALU.add)
                nc.sync.dma_start(out=of[ch0:ch0 + psize, c0:c0 + n],
                                  in_=pt[pbase:pbase + psize, :n])

            for e, o in ((0, 1), (2, 3)):
                (ce, ne), (co_, no_) = cts[e], cts[o]
                p1 = psO.tile([128, CT], F32, tag="po")
                mm_group2(p1, [((0, 32), (0, 32), ce, ne, (0, 0)),
                               ((32, 64), (32, 64), ce, ne, (0, 32)),
                               ((64, 96), (64, 96), ce, ne, (0, 64)),
                               ((96, 128), (0, 32), co_, no_, (0, 96))])
                aligned_epi(p1, ce, ne)
                piece_epi(p1, 96, 32, 0, co_, no_)
            p2 = psG.tile([128, CT], F32, tag="pg")
            mm_group2(p2, [((0, 32), (32, 64), cts[1][0], cts[1][1], (0, 0)),
                           ((32, 64), (64, 96), cts[1][0], cts[1][1], (0, 32)),
                           ((64, 96), (32, 64), cts[3][0], cts[3][1], (0, 64)),
                           ((96, 128), (64, 96), cts[3][0], cts[3][1], (0, 96))])
            piece_epi(p2, 0, 64, 32, cts[1][0], cts[1][1])
            piece_epi(p2, 64, 64, 32, cts[3][0], cts[3][1])
            p4 = psO.tile([128, CT], F32, tag="po")
            mm_group(p4, (0, 96), (0, 96), cts[4][0], cts[4][1], (0, 0))
            aligned_epi(p4, cts[4][0], cts[4][1])
            nc.sync.dma_start(out=out[:, t], in_=ot[:, :, 1:1 + W])

        octs = col_tiles(NO)
        use_p2 = pack2 and len(octs) == 5 and all(n == CT for _, n in octs[:4])
        co_fn = conv_out_packed if use_p2 else conv_out

        stage_a(0)
        if T > 1:
            stage_a(1)
        for t in range(T):
            if t + 1 < T:
                stage_a(t + 1)
            bt = conv_in_packed(t) if pack else conv_in(t)
            scan_step(t, bt)
            if t >= 1:
                co_fn(t - 1)
        co_fn(T - 1)

    nc.compile()
    return nc


def prep_core_inputs(x, norm_w, conv_in_w, conv_in_b, nu_log, theta_log,
                     c_re, c_im, conv_out_w, conv_out_b, n_qh):
    """Build per-core input maps. Cores = batch-major, then H quarters."""
    B, C, T, H, W = x.shape
    HR = H // n_qh

    nu = np.exp(np.asarray(nu_log, np.float64))
    theta = np.exp(np.asarray(theta_log, np.float64))
    lam_re = (np.exp(-nu) * np.cos(theta)).astype(np.float32)
    lam_im = (np.exp(-nu) * np.sin(theta)).astype(np.float32)
    gamma = np.sqrt(1.0 - np.exp(-2.0 * nu))
    gcre = (gamma * np.asarray(c_re, np.float64)).astype(np.float32)
    gcim = (gamma * np.asarray(c_im, np.float64)).astype(np.float32)

    w_in_f = np.asarray(conv_in_w, np.float32) * \
        np.asarray(norm_w, np.float32)[None, :, None, None, None]
    w_in_t = np.ascontiguousarray(
        np.transpose(w_in_f, (1, 2, 3, 4, 0)).reshape(C, 27, 2 * C)
    ).astype(ml_dtypes.bfloat16)
    w_out_t = np.ascontiguousarray(
        np.transpose(np.asarray(conv_out_w, np.float32),
                     (1, 2, 3, 4, 0)).reshape(C, 27, C)
    ).astype(ml_dtypes.bfloat16)
    ones = np.ones((C, 128), ml_dtypes.bfloat16)

    xp = np.concatenate([x[..., -1:], x, x[..., :1]], axis=-1)  # W circular

    in_maps = []
    for b in range(B):
        for q in range(n_qh):
            rows = np.clip(np.arange(q * HR - 2, q * HR + HR + 2), 0, H - 1)
            xh = np.ascontiguousarray(xp[b][:, :, rows, :]).astype(np.float32)
            m0 = 0.0 if q == 0 else 1.0
            m1 = 0.0 if q == n_qh - 1 else 1.0
            cvec = np.stack([
                np.asarray(conv_in_b, np.float32)[:C],
                np.asarray(conv_in_b, np.float32)[C:],
                lam_re, lam_im, -lam_im, gcre, gcim,
                np.asarray(conv_out_b, np.float32),
                np.full(C, m0, np.float32), np.full(C, 1.0 - m0, np.float32),
                np.full(C, m1, np.float32), np.full(C, 1.0 - m1, np.float32),
                np.full(C, EPS, np.float32),
            ], axis=1)
            bo = np.asarray(conv_out_b, np.float32)
            aux = np.zeros((128, 2), np.float32)
            aux[96:128, 0] = bo[0:32]
            aux[:, 1] = bo[32 + (np.arange(128) % 64)]
            in_maps.append({
                "xh": xh,
                "w_in": w_in_t,
                "w_out": w_out_t,
                "onesw": ones,
                "consts": np.ascontiguousarray(cvec),
                "consts2": aux,
            })
    return in_maps


LAST_RESULT = None  # BassKernelResults of the most recent kernel() call


def kernel(x, norm_w, conv_in_w, conv_in_b, nu_log, theta_log, c_re, c_im,
           conv_out_w, conv_out_b):
    global LAST_RESULT
    from concourse.bass_utils import run_bass_kernel_spmd

    x = np.asarray(x, np.float32)
    B, C, T, H, W = x.shape
    HR = H // QH
    in_maps = prep_core_inputs(x, norm_w, conv_in_w, conv_in_b, nu_log,
                               theta_log, c_re, c_im, conv_out_w, conv_out_b,
                               QH)
    nc = build_program(C=C, T=T, HR=HR, W=W, CT=512,
                       use_silu=os.environ.get("KERNEL_NO_SILU", "") != "1",
                       pack=os.environ.get("KERNEL_PACK", "1") == "1",
                       pack2=os.environ.get("KERNEL_PACK2", "0") == "1")
    trace = os.environ.get("KERNEL_TRACE", "") == "1"
    res = run_bass_kernel_spmd(nc, in_maps, list(range(N_CORES)), trace=trace)
    LAST_RESULT = res
    out = np.empty((B, C, T, H, W), np.float32)
    for core in range(N_CORES):
        b, q = core // QH, core % QH
        out[b, :, :, q * HR:(q + 1) * HR, :] = res.results[core]["out"]
    return out



# revision 5
# speedup vs baseline: 1.0033x; 1.0033x over previous
"""ConvLRUBlock Trainium2 kernel.

Reference computation (per batch b):
    h   = rms_norm(x, norm_w)                  # over channel dim
    uv  = conv3d_3x3x3(h, w_in) + b_in         # pad: replicate T/H, circular W
    u   = silu(a) * g          (a, g = uv split on channels)
    y_t = Re(h_t) c_re + Im(h_t) c_im,  h_t = lam h_{t-1} + gamma u_t  (diag LRU)
    out = x + conv3d_3x3x3(y, w_out) + b_out

Sharding: 8 cores = (batch 2) x (H quarters 4). Each core receives its H
slice plus 2 halo rows each side (edge-replicated) and the W dim circularly
padded to W+2, so no inter-core communication is needed.

In-kernel layout: channels (96) on SBUF partitions; spatial (rows x (W+2))
flattened on the free dim.

conv_in runs as bf16 matmuls with K=128 partition augmentation: partitions
96:128 of each hn plane hold a pre-shifted copy of one 32-channel group
(4 shift configs as planes), so each pass contracts one full tap (96 ch)
plus one (tap, ch-group) unit of a leftover tap: 7 passes/slab instead of 9
(42 vs 54 streamed passes per t).

conv_out runs as fp8-e4m3 DoubleRow matmuls pairing two taps per pass
(w[0]*y[tap1] + w[1]*y[tap2] per partition): 5 passes/slab instead of 9
(15 vs 27 per t). Weights are scaled by 1024 (e4m3 subnormal avoidance);
the epilogue divides via a per-partition scalar. Set out_fp8=False for a
plain bf16 conv_out (fallback if the fp8 accuracy is not acceptable).

The LRU scan is 16 sequential complex steps on the vector engine.
"""

import os
from contextlib import ExitStack

import ml_dtypes
import numpy as np

import concourse.bacc as bacc
import concourse.bass as bass  # noqa: F401
import concourse.tile as tile
from concourse import mybir

F32 = mybir.dt.float32
BF16 = mybir.dt.bfloat16
FP8 = mybir.dt.float8e4
ALU = mybir.AluOpType
AF = mybir.ActivationFunctionType
E4NP = ml_dtypes.float8_e4m3fn

EPS = 1e-6
W_SCALE = 1024.0  # fp8 weight upscale

# Full-problem constants
B_FULL, C_FULL, T_FULL, H_FULL, W_FULL = 2, 96, 16, 64, 128
QH = 4  # H quarters
N_CORES = 8

N_CFG = 4   # hn aug shift configs (planes)
N_PASS = 7  # conv_in passes per kt-slab
N_PAIR = 5  # conv_out DoubleRow passes per kt-slab


def make_schedule(Wp):
    """conv_in K=128 augmentation schedule (per kt-slab).

    Pass list entries: ((kh, kw), cfg, covered-tap-or-None). Partitions
    96:128 of hn plane `cfg` hold ch-group CFGS[cfg][0] pre-shifted by
    CFGS[cfg][1] columns, so the pass also contracts that group of the
    covered leftover tap. Leftover taps: (2,1), (2,2)."""
    cfgs = [(0, Wp), (1, 2 * Wp), (2, Wp + 1), (2, 2 * Wp + 2)]  # (grp, rho)
    passes = [
        ((0, 0), 3, (2, 2)),
        ((0, 1), 1, (2, 1)),
        ((0, 2), 1, (2, 2)),
        ((1, 0), 2, (2, 1)),
        ((1, 1), 0, (2, 1)),
        ((1, 2), 0, (2, 2)),
        ((2, 0), 0, None),  # aug slot zero-weighted
    ]
    covered = set()
    for (kh, kw), cfg, tapb in passes:
        if tapb is None:
            continue
        g, rho = cfgs[cfg]
        assert (tapb[0] * Wp + tapb[1]) - (kh * Wp + kw) == rho
        covered.add((tapb, g))
    assert len(covered) == 6
    return cfgs, passes


# conv_out DoubleRow tap pairs (within a kt-slab); deltas must be even.
OUT_PAIRS = [
    ((0, 0), (0, 2)),
    ((1, 0), (1, 2)),
    ((2, 0), (2, 2)),
    ((0, 1), (1, 1)),
    ((2, 1), None),  # second slot zero-weighted
]


def build_program(C=96, T=16, HR=16, W=128, CT=512, use_silu=True,
                  out_fp8=True):
    """Build the single-core SPMD Bass program. use_silu: Silu on ACT vs
    Sigmoid+mults (the simulator does not implement Silu)."""
    Wp = W + 2           # circular-padded width
    RIN = HR + 4         # input rows (2 halo each side, for two convs)
    RU = HR + 2          # u/y rows (1 halo each side, for conv_out)
    NIN = RIN * Wp       # flattened input cols per t
    NU = RU * Wp         # flattened u/y cols per t
    NO = HR * Wp         # flattened output cols per t
    NPL = 1 + NIN + 1    # hn plane width
    NPLU = 1 + NU + 1    # y plane width
    YDT = FP8 if out_fp8 else BF16

    CFGS, PASSES = make_schedule(Wp)

    nc = bacc.Bacc()
    xh = nc.declare_dram_parameter("xh", [C, T, RIN, Wp], F32, isOutput=False)
    w_in = nc.declare_dram_parameter("w_in", [128, 3, N_PASS, 2 * C], BF16,
                                     isOutput=False)
    if out_fp8:
        w_out = nc.declare_dram_parameter("w_out", [C, 3, N_PAIR, 2, C], FP8,
                                          isOutput=False)
    else:
        w_out = nc.declare_dram_parameter("w_out", [C, 27, C], BF16,
                                          isOutput=False)
    onesw = nc.declare_dram_parameter("onesw", [C, 128], BF16, isOutput=False)
    consts = nc.declare_dram_parameter("consts", [C, 14], F32, isOutput=False)
    out = nc.declare_dram_parameter("out", [C, T, HR, W], F32, isOutput=True)

    def col_tiles(total):
        return [(i, min(CT, total - i)) for i in range(0, total, CT)]

    with tile.TileContext(nc) as tc, ExitStack() as ctx:
        singles = ctx.enter_context(tc.tile_pool(name="singles", bufs=1))
        xpool = ctx.enter_context(tc.tile_pool(name="xpool", bufs=2))
        sqpool = ctx.enter_context(tc.tile_pool(name="sqpool", bufs=2))
        statpool = ctx.enter_context(tc.tile_pool(name="statpool", bufs=2))
        hnpool = ctx.enter_context(tc.tile_pool(name="hnpool", bufs=3))
        sapool = ctx.enter_context(tc.tile_pool(name="sapool", bufs=3))
        bpool = ctx.enter_context(tc.tile_pool(name="bpool", bufs=2))
        hrpool = ctx.enter_context(tc.tile_pool(name="hrpool", bufs=2))
        hipool = ctx.enter_context(tc.tile_pool(name="hipool", bufs=2))
        tmppool = ctx.enter_context(tc.tile_pool(name="tmppool", bufs=1))
        ypool = ctx.enter_context(tc.tile_pool(name="ypool", bufs=3))
        opool = ctx.enter_context(tc.tile_pool(name="opool", bufs=2))
        touchpool = ctx.enter_context(tc.tile_pool(name="touchpool", bufs=2))
        psN = ctx.enter_context(tc.tile_pool(name="psN", bufs=2, space="PSUM"))
        psA = ctx.enter_context(tc.tile_pool(name="psA", bufs=2, space="PSUM"))
        psG = ctx.enter_context(tc.tile_pool(name="psG", bufs=2, space="PSUM"))
        psO = ctx.enter_context(tc.tile_pool(name="psO", bufs=2, space="PSUM"))

        sb_win = singles.tile([128, 3, N_PASS, 2 * C], BF16)
        nc.sync.dma_start(out=sb_win[:], in_=w_in[:])
        if out_fp8:
            sb_wout = singles.tile([C, 3, N_PAIR, 2, C], FP8)
        else:
            sb_wout = singles.tile([C, 27, C], BF16)
        nc.sync.dma_start(out=sb_wout[:], in_=w_out[:])
        sb_ones = singles.tile([C, 128], BF16)
        nc.sync.dma_start(out=sb_ones[:], in_=onesw[:])
        sb_c = singles.tile([C, 14], F32)
        nc.sync.dma_start(out=sb_c[:], in_=consts[:])
        c_ba = sb_c[:, 0:1]
        c_bg = sb_c[:, 1:2]
        c_lr = sb_c[:, 2:3]
        c_li = sb_c[:, 3:4]
        c_nli = sb_c[:, 4:5]
        c_gcre = sb_c[:, 5:6]
        c_gcim = sb_c[:, 6:7]
        c_bout = sb_c[:, 7:8]
        c_m0 = sb_c[:, 8:9]
        c_1m0 = sb_c[:, 9:10]
        c_m1 = sb_c[:, 10:11]
        c_1m1 = sb_c[:, 11:12]
        c_eps = sb_c[:, 12:13]
        c_wsc = sb_c[:, 13:14]  # 1/W_SCALE (fp8) or 1.0

        # Warm-up reads: make each compute engine observe the const-DMA
        # semaphores early, so steady-state ops carry at most one sync wait
        # (walrus rejects DVE ops with two wait commands).
        wu_v = singles.tile([C, 14], F32)
        nc.vector.tensor_copy(wu_v[:], sb_c[:])
        wu_s = singles.tile([C, 14], F32)
        nc.scalar.activation(wu_s[:], sb_c[:], AF.Square)

        def touch(ap, engines="v"):
            """Tiny read of a freshly-DMA'd tile so the engine observes the
            DMA-queue semaphore here; later big consumers then carry only
            engine-sem waits (walrus rejects DVE ops with 2 sync waits)."""
            if "v" in engines:
                tv = touchpool.tile([C, 1], F32, tag="tv")
                nc.vector.tensor_copy(tv[:], ap)
            if "s" in engines:
                ts_ = touchpool.tile([C, 1], F32, tag="ts")
                nc.scalar.activation(ts_[:], ap, AF.Square)

        hn_slabs = [None] * T  # hnorm tiles [128, N_CFG, NPL], data at col 1
        y_slabs = [None] * T   # y tiles [C, NPLU], data at col 1

        def stage_a(t):
            """x[t] -> hnorm[t] (rms-normed, bf16, [128, N_CFG, NPL])."""
            xt = xpool.tile([C, RIN, Wp], F32, tag="xt")
            nc.sync.dma_start(out=xt[:], in_=xh[:, t])
            touch(xt[:, 0, 0:1], engines="vs")
            xf = xt[:].rearrange("p r w -> p (r w)")
            hn = hnpool.tile([128, N_CFG, NPL], BF16, tag="hn")
            for p in range(N_CFG):
                nc.vector.memset(hn[0:C, p, 0:1], 0.0)
                nc.vector.memset(hn[0:C, p, 1 + NIN:], 0.0)
            for c0, n in col_tiles(NIN):
                sq = sqpool.tile([C, CT], BF16, tag="sq")
                nc.scalar.activation(sq[:, :n], xf[:, c0:c0 + n], AF.Square)
                ps = psN.tile([128, CT], F32, tag="psn")
                nc.tensor.matmul(ps[:, :n], sb_ones[:], sq[:, :n],
                                 start=True, stop=True)
                lg = statpool.tile([C, CT], F32, tag="lg")
                nc.scalar.activation(lg[:, :n], ps[:C, :n], AF.Ln,
                                     scale=1.0 / C, bias=c_eps)
                inv = statpool.tile([C, CT], F32, tag="inv")
                nc.scalar.activation(inv[:, :n], lg[:, :n], AF.Exp, scale=-0.5)
                for p in range(N_CFG):
                    nc.vector.tensor_mul(hn[0:C, p, 1 + c0:1 + c0 + n],
                                         xf[:, c0:c0 + n], inv[:, :n])
            # partitions 96:128 of each plane: pre-shifted 32-ch group copy
            for c, (g, rho) in enumerate(CFGS):
                nc.sync.dma_start(
                    out=hn[96:128, c, 0:NPL - rho],
                    in_=hn[g * 32:(g + 1) * 32, 0, rho:NPL])
                nc.vector.memset(hn[96:128, c, NPL - rho:NPL], 0.0)
            hn_slabs[t] = hn
            return hn

        def gate_epilogue(pa, pg, bt, c0, n):
            """silu(a+ba)*(g+bg) for one coltile; a=pa[0:C], g=pg[0:C]."""
            if use_silu:
                sa = sapool.tile([C, CT], BF16, tag="sa")
                nc.scalar.activation(sa[:, :n], pa[:C, :n], AF.Silu, bias=c_ba)
            else:
                sg = sapool.tile([C, CT], BF16, tag="sg")
                nc.scalar.activation(sg[:, :n], pa[:C, :n], AF.Sigmoid,
                                     bias=c_ba)
                av = sapool.tile([C, CT], F32, tag="av")
                nc.vector.scalar_tensor_tensor(av[:, :n], pa[:C, :n], c_ba,
                                               sg[:, :n], ALU.add, ALU.bypass)
                sa = sapool.tile([C, CT], BF16, tag="sa")
                nc.vector.tensor_mul(sa[:, :n], sg[:, :n], av[:, :n])
            nc.vector.scalar_tensor_tensor(bt[:, c0:c0 + n], pg[:C, :n],
                                           c_bg, sa[:, :n],
                                           ALU.add, ALU.mult)

        def conv_in(t):
            """hnorm[t-1..t+1] -> b[t] = silu(a+ba)*(g+bg), bf16 [C, NU]."""
            slabs = [hn_slabs[min(max(t + kt - 1, 0), T - 1)] for kt in range(3)]
            bt = bpool.tile([C, NU], BF16, tag="bt")
            n_mm = 3 * N_PASS
            for c0, n in col_tiles(NU):
                pa = psA.tile([C, CT], F32, tag="pa")
                pg = psG.tile([C, CT], F32, tag="pg")
                for half, ps in ((0, pa), (1, pg)):
                    idx = 0
                    for kt in range(3):
                        slab = slabs[kt]
                        for p, ((kh, kw), cfg, _tapb) in enumerate(PASSES):
                            s = c0 + kh * Wp + kw
                            rhs = slab[:, cfg, s:s + n]
                            nc.tensor.matmul(
                                ps[:, :n],
                                sb_win[:, kt, p, half * C:(half + 1) * C],
                                rhs, start=(idx == 0), stop=(idx == n_mm - 1))
                            idx += 1
                gate_epilogue(pa, pg, bt, c0, n)
            return bt

        scan_state = [None, None]  # hr, hi tiles [C, NU] f32

        def scan_step(t, bt):
            """LRU step + projection -> y[t] ([C, NPLU], fp8 or bf16)."""
            hr_new = hrpool.tile([C, NU], F32, tag="hr")
            hi_new = hipool.tile([C, NU], F32, tag="hi")
            if t == 0:
                nc.vector.tensor_copy(hr_new[:], bt[:])
                nc.vector.memset(hi_new[:], 0.0)
            else:
                hr_old, hi_old = scan_state
                t1 = tmppool.tile([C, NU], F32, tag="tA")
                nc.vector.scalar_tensor_tensor(t1[:], hi_old[:], c_nli, bt[:],
                                               ALU.mult, ALU.add)
                nc.vector.scalar_tensor_tensor(hr_new[:], hr_old[:], c_lr,
                                               t1[:], ALU.mult, ALU.add)
                t2 = tmppool.tile([C, NU], F32, tag="tB")
                nc.vector.scalar_tensor_tensor(t2[:], hi_old[:], c_lr,
                                               hi_old[:], ALU.mult, ALU.bypass)
                nc.vector.scalar_tensor_tensor(hi_new[:], hr_old[:], c_li,
                                               t2[:], ALU.mult, ALU.add)
            scan_state[0], scan_state[1] = hr_new, hi_new
            t3 = tmppool.tile([C, NU], F32, tag="tA")
            nc.vector.scalar_tensor_tensor(t3[:], hr_new[:], c_gcre,
                                           hr_new[:], ALU.mult, ALU.bypass)
            yt = ypool.tile([C, NPLU], YDT, tag="yt")
            nc.vector.memset(yt[:, 0:1], 0.0)
            nc.vector.memset(yt[:, 1 + NU:], 0.0)
            nc.vector.scalar_tensor_tensor(yt[:, 1:1 + NU], hi_new[:],
                                           c_gcim, t3[:], ALU.mult, ALU.add)
            # W wrap columns: col 0 <- col W (w=W-1), col W+1 <- col 1 (w=0)
            yv = yt[:, 1:1 + NU].rearrange("p (r w) -> p r w", w=Wp)
            nc.vector.tensor_copy(yv[:, :, 0:1], yv[:, :, W:W + 1])
            nc.vector.tensor_copy(yv[:, :, W + 1:W + 2], yv[:, :, 1:2])
            # H edge replication (active only on global-edge cores, via mask):
            # row0 <- m0*row0 + (1-m0)*row1 ; last <- m1*last + (1-m1)*prev
            e0 = tmppool.tile([C, Wp], F32, tag="tE")
            nc.vector.scalar_tensor_tensor(e0[:], yv[:, 1, :], c_1m0,
                                           yv[:, 1, :], ALU.mult, ALU.bypass)
            nc.vector.scalar_tensor_tensor(yv[:, 0, :], yv[:, 0, :], c_m0,
                                           e0[:], ALU.mult, ALU.add)
            e1 = tmppool.tile([C, Wp], F32, tag="tE")
            nc.vector.scalar_tensor_tensor(e1[:], yv[:, RU - 2, :], c_1m1,
                                           yv[:, RU - 2, :], ALU.mult, ALU.bypass)
            nc.vector.scalar_tensor_tensor(yv[:, RU - 1, :], yv[:, RU - 1, :],
                                           c_m1, e1[:], ALU.mult, ALU.add)
            y_slabs[t] = yt
            return yt

        def conv_out(t):
            """y[t-1..t+1] -> out[t] = x + conv(y) + b_out."""
            slabs = [y_slabs[min(max(t + kt - 1, 0), T - 1)] for kt in range(3)]
            ot = opool.tile([C, HR, Wp], F32, tag="ot")
            # residual input loaded into the output staging tile
            nc.sync.dma_start(out=ot[:], in_=xh[:, t, 2:2 + HR, :])
            touch(ot[:, 0, 0:1], engines="v")
            of = ot[:].rearrange("p r w -> p (r w)")
            if out_fp8:
                # pre-add b_out to the residual staging tile
                nc.vector.scalar_tensor_tensor(of[:, :], of[:, :], c_bout,
                                               of[:, :], ALU.add, ALU.bypass)
                n_mm = 3 * N_PAIR
                for c0, n in col_tiles(NO):
                    po = psO.tile([C, CT], F32, tag="po")
                    idx = 0
                    for kt in range(3):
                        slab = slabs[kt]
                        for p, (tap1, tap2) in enumerate(OUT_PAIRS):
                            d1 = tap1[0] * Wp + tap1[1]
                            d2 = (tap2[0] * Wp + tap2[1]) if tap2 else d1
                            rhs = slab[:, c0 + d1:c0 + d1 + n].unsqueeze(1)
                            rhs.ap[1] = (d2 - d1, 2)
                            rhs.ap[2] = (1, n)
                            nc.tensor.matmul(
                                po[:, :n], sb_wout[:, kt, p], rhs,
                                start=(idx == 0), stop=(idx == n_mm - 1),
                                perf_mode=mybir.MatmulPerfMode.DoubleRow)
                            idx += 1
                    nc.vector.scalar_tensor_tensor(of[:, c0:c0 + n],
                                                   po[:, :n], c_wsc,
                                                   of[:, c0:c0 + n],
                                                   ALU.mult, ALU.add)
            else:
                n_mm = 27
                for c0, n in col_tiles(NO):
                    po = psO.tile([C, CT], F32, tag="po")
                    idx = 0
                    for kt in range(3):
                        slab = slabs[kt]
                        for kh in range(3):
                            for kw in range(3):
                                s = c0 + kh * Wp + kw
                                nc.tensor.matmul(
                                    po[:, :n], sb_wout[:, idx % 27, :],
                                    slab[:, s:s + n],
                                    start=(idx == 0), stop=(idx == n_mm - 1))
                                idx += 1
                    nc.vector.scalar_tensor_tensor(of[:, c0:c0 + n],
                                                   po[:, :n], c_bout,
                                                   of[:, c0:c0 + n],
                                                   ALU.add, ALU.add)
            nc.sync.dma_start(out=out[:, t], in_=ot[:, :, 1:1 + W])

        stage_a(0)
        if T > 1:
            stage_a(1)
        for t in range(T):
            if t + 1 < T:
                stage_a(t + 1)
            bt = conv_in(t)
            scan_step(t, bt)
            if t >= 1:
                conv_out(t - 1)
        conv_out(T - 1)

    nc.compile()
    return nc


def prep_weight_aug(w_t, Wp, n_out):
    """Pack [C, 27, n_out] tap-major weights into the augmented layout
    [128, 3, N_PASS, n_out]: rows 0:96 = base tap A, rows 96:128 = the
    covered leftover unit's 32-channel group (or zero)."""
    C = w_t.shape[0]
    CFGS, PASSES = make_schedule(Wp)
    w_aug = np.zeros((128, 3, N_PASS, n_out), np.float32)
    for kt in range(3):
        for p, ((kh, kw), cfg, tapb) in enumerate(PASSES):
            off = kt * 9 + kh * 3 + kw
            w_aug[0:C, kt, p] = w_t[:, off]
            if tapb is not None:
                g, _rho = CFGS[cfg]
                off_b = kt * 9 + tapb[0] * 3 + tapb[1]
                w_aug[96:128, kt, p] = w_t[g * 32:(g + 1) * 32, off_b]
    return w_aug.astype(ml_dtypes.bfloat16)


def prep_weight_pairs_fp8(w_t, n_out):
    """Pack [C, 27, n_out] tap-major weights into DoubleRow pair layout
    [C, 3, N_PAIR, 2, n_out] fp8, scaled by W_SCALE."""
    C = w_t.shape[0]
    w_p = np.zeros((C, 3, N_PAIR, 2, n_out), np.float32)
    for kt in range(3):
        for p, (tap1, tap2) in enumerate(OUT_PAIRS):
            w_p[:, kt, p, 0] = w_t[:, kt * 9 + tap1[0] * 3 + tap1[1]]
            if tap2 is not None:
                w_p[:, kt, p, 1] = w_t[:, kt * 9 + tap2[0] * 3 + tap2[1]]
    return np.clip(w_p * W_SCALE, -240.0, 240.0).astype(E4NP)


def prep_core_inputs(x, norm_w, conv_in_w, conv_in_b, nu_log, theta_log,
                     c_re, c_im, conv_out_w, conv_out_b, n_qh, out_fp8=True):
    """Build per-core input maps. Cores = batch-major, then H quarters."""
    B, C, T, H, W = x.shape
    HR = H // n_qh
    Wp = W + 2

    nu = np.exp(np.asarray(nu_log, np.float64))
    theta = np.exp(np.asarray(theta_log, np.float64))
    lam_re = (np.exp(-nu) * np.cos(theta)).astype(np.float32)
    lam_im = (np.exp(-nu) * np.sin(theta)).astype(np.float32)
    gamma = np.sqrt(1.0 - np.exp(-2.0 * nu))
    gcre = (gamma * np.asarray(c_re, np.float64)).astype(np.float32)
    gcim = (gamma * np.asarray(c_im, np.float64)).astype(np.float32)

    w_in_f = np.asarray(conv_in_w, np.float32) * \
        np.asarray(norm_w, np.float32)[None, :, None, None, None]
    w_in_t = np.ascontiguousarray(
        np.transpose(w_in_f, (1, 2, 3, 4, 0)).reshape(C, 27, 2 * C))
    w_out_t = np.ascontiguousarray(
        np.transpose(np.asarray(conv_out_w, np.float32),
                     (1, 2, 3, 4, 0)).reshape(C, 27, C))
    w_in_aug = prep_weight_aug(w_in_t, Wp, 2 * C)
    if out_fp8:
        w_out_k = prep_weight_pairs_fp8(w_out_t, C)
        wsc = np.full(C, 1.0 / W_SCALE, np.float32)
    else:
        w_out_k = w_out_t.astype(ml_dtypes.bfloat16)
        wsc = np.ones(C, np.float32)
    ones = np.ones((C, 128), ml_dtypes.bfloat16)

    xp = np.concatenate([x[..., -1:], x, x[..., :1]], axis=-1)  # W circular

    in_maps = []
    for b in range(B):
        for q in range(n_qh):
            rows = np.clip(np.arange(q * HR - 2, q * HR + HR + 2), 0, H - 1)
            xh = np.ascontiguousarray(xp[b][:, :, rows, :]).astype(np.float32)
            m0 = 0.0 if q == 0 else 1.0
            m1 = 0.0 if q == n_qh - 1 else 1.0
            cvec = np.stack([
                np.asarray(conv_in_b, np.float32)[:C],
                np.asarray(conv_in_b, np.float32)[C:],
                lam_re, lam_im, -lam_im, gcre, gcim,
                np.asarray(conv_out_b, np.float32),
                np.full(C, m0, np.float32), np.full(C, 1.0 - m0, np.float32),
                np.full(C, m1, np.float32), np.full(C, 1.0 - m1, np.float32),
                np.full(C, EPS, np.float32),
                wsc,
            ], axis=1)
            in_maps.append({
                "xh": xh,
                "w_in": w_in_aug,
                "w_out": w_out_k,
                "onesw": ones,
                "consts": np.ascontiguousarray(cvec),
            })
    return in_maps


LAST_RESULT = None  # BassKernelResults of the most recent kernel() call


def kernel(x, norm_w, conv_in_w, conv_in_b, nu_log, theta_log, c_re, c_im,
           conv_out_w, conv_out_b):
    global LAST_RESULT
    from concourse.bass_utils import run_bass_kernel_spmd

    x = np.asarray(x, np.float32)
    B, C, T, H, W = x.shape
    HR = H // QH
    out_fp8 = os.environ.get("KERNEL_OUT_FP8", "1") == "1"
    in_maps = prep_core_inputs(x, norm_w, conv_in_w, conv_in_b, nu_log,
                               theta_log, c_re, c_im, conv_out_w, conv_out_b,
                               QH, out_fp8=out_fp8)
    nc = build_program(C=C, T=T, HR=HR, W=W, CT=512, out_fp8=out_fp8)
    trace = os.environ.get("KERNEL_TRACE", "") == "1"
    res = run_bass_kernel_spmd(nc, in_maps, list(range(N_CORES)), trace=trace)
    LAST_RESULT = res
    out = np.empty((B, C, T, H, W), np.float32)
    for core in range(N_CORES):
        b, q = core // QH, core % QH
        out[b, :, :, q * HR:(q + 1) * HR, :] = res.results[core]["out"]
    return out


# revision 8
# speedup vs baseline: 1.1625x; 1.1586x over previous
"""ConvLRUBlock Trainium2 kernel.

Reference computation (per batch b):
    h   = rms_norm(x, norm_w)                  # over channel dim
    uv  = conv3d_3x3x3(h, w_in) + b_in         # pad: replicate T/H, circular W
    u   = silu(a) * g          (a, g = uv split on channels)
    y_t = Re(h_t) c_re + Im(h_t) c_im,  h_t = lam h_{t-1} + gamma u_t  (diag LRU)
    out = x + conv3d_3x3x3(y, w_out) + b_out

Sharding: 8 cores = (batch 2) x (H quarters 4). Each core receives its H
slice plus 2 halo rows each side (edge-replicated) and the W dim circularly
padded to W+2, so no inter-core communication is needed.

In-kernel layout: channels (96) on SBUF partitions; spatial (rows x (W+2))
flattened on the free dim.

conv_in runs as bf16 matmuls with K=128 partition augmentation: partitions
96:128 of each hn plane hold a pre-shifted copy of one 32-channel group
(4 shift configs as planes), so each pass contracts one full tap (96 ch)
plus one (tap, ch-group) unit of a leftover tap: 7 passes/slab instead of 9
(42 vs 54 streamed passes per t).

conv_out runs as fp8-e4m3 DoubleRow matmuls pairing two taps per pass
(w[0]*y[tap1] + w[1]*y[tap2] per partition): 5 passes/slab instead of 9
(15 vs 27 per t). Weights are scaled by 1024 (e4m3 subnormal avoidance);
the epilogue divides via a per-partition scalar. Set out_fp8=False for a
plain bf16 conv_out (fallback if the fp8 accuracy is not acceptable).

The LRU scan is 16 sequential complex steps on the vector engine.
"""

import os
from contextlib import ExitStack

import ml_dtypes
import numpy as np

import concourse.bacc as bacc
import concourse.bass as bass  # noqa: F401
import concourse.tile as tile
from concourse import mybir

F32 = mybir.dt.float32
BF16 = mybir.dt.bfloat16
FP8 = mybir.dt.float8e4
ALU = mybir.AluOpType
AF = mybir.ActivationFunctionType
E4NP = ml_dtypes.float8_e4m3fn

EPS = 1e-6
W_SCALE = 1024.0  # fp8 weight upscale

# Full-problem constants
B_FULL, C_FULL, T_FULL, H_FULL, W_FULL = 2, 96, 16, 64, 128
QH = 4  # H quarters
N_CORES = 8

N_CFG = 4   # hn aug shift configs (planes)
N_PASS = 7  # conv_in passes per kt-slab
N_PAIR = 5  # conv_out DoubleRow passes per kt-slab


def make_schedule(Wp):
    """conv_in K=128 augmentation schedule (per kt-slab).

    Pass list entries: ((kh, kw), cfg, covered-tap-or-None). Partitions
    96:128 of hn plane `cfg` hold ch-group CFGS[cfg][0] pre-shifted by
    CFGS[cfg][1] columns, so the pass also contracts that group of the
    covered leftover tap. Leftover taps: (2,1), (2,2)."""
    cfgs = [(0, Wp), (1, 2 * Wp), (2, Wp + 1), (2, 2 * Wp + 2)]  # (grp, rho)
    passes = [
        ((0, 0), 3, (2, 2)),
        ((0, 1), 1, (2, 1)),
        ((0, 2), 1, (2, 2)),
        ((1, 0), 2, (2, 1)),
        ((1, 1), 0, (2, 1)),
        ((1, 2), 0, (2, 2)),
        ((2, 0), 0, None),  # aug slot zero-weighted
    ]
    covered = set()
    for (kh, kw), cfg, tapb in passes:
        if tapb is None:
            continue
        g, rho = cfgs[cfg]
        assert (tapb[0] * Wp + tapb[1]) - (kh * Wp + kw) == rho
        covered.add((tapb, g))
    assert len(covered) == 6
    return cfgs, passes


# conv_out DoubleRow tap pairs (within a kt-slab); deltas must be even.
OUT_PAIRS = [
    ((0, 0), (0, 2)),
    ((1, 0), (1, 2)),
    ((2, 0), (2, 2)),
    ((0, 1), (1, 1)),
    ((2, 1), None),  # second slot zero-weighted
]


def build_program(C=96, T=16, HR=16, W=128, CT=512, use_silu=True,
                  out_fp8=True):
    """Build the single-core SPMD Bass program. use_silu: Silu on ACT vs
    Sigmoid+mults (the simulator does not implement Silu)."""
    Wp = W + 2           # circular-padded width
    RIN = HR + 4         # input rows (2 halo each side, for two convs)
    RU = HR + 2          # u/y rows (1 halo each side, for conv_out)
    NIN = RIN * Wp       # flattened input cols per t
    NU = RU * Wp         # flattened u/y cols per t
    NO = HR * Wp         # flattened output cols per t
    NPL = 1 + NIN + 1    # hn plane width
    NPLU = 1 + NU + 1    # y plane width
    YDT = FP8 if out_fp8 else BF16

    CFGS, PASSES = make_schedule(Wp)

    nc = bacc.Bacc()
    xh = nc.declare_dram_parameter("xh", [C, T, RIN, Wp], F32, isOutput=False)
    w_in = nc.declare_dram_parameter("w_in", [128, 3, N_PASS, 2 * C], BF16,
                                     isOutput=False)
    if out_fp8:
        w_out = nc.declare_dram_parameter("w_out", [C, 3, N_PAIR, 2, C], FP8,
                                          isOutput=False)
    else:
        w_out = nc.declare_dram_parameter("w_out", [C, 27, C], BF16,
                                          isOutput=False)
    onesw = nc.declare_dram_parameter("onesw", [C, 128], BF16, isOutput=False)
    consts = nc.declare_dram_parameter("consts", [C, 14], F32, isOutput=False)
    out = nc.declare_dram_parameter("out", [C, T, HR, W], F32, isOutput=True)

    def col_tiles(total):
        return [(i, min(CT, total - i)) for i in range(0, total, CT)]

    with tile.TileContext(nc) as tc, ExitStack() as ctx:
        singles = ctx.enter_context(tc.tile_pool(name="singles", bufs=1))
        xpool = ctx.enter_context(tc.tile_pool(name="xpool", bufs=2))
        sqpool = ctx.enter_context(tc.tile_pool(name="sqpool", bufs=2))
        statpool = ctx.enter_context(tc.tile_pool(name="statpool", bufs=2))
        hnpool = ctx.enter_context(tc.tile_pool(name="hnpool", bufs=4))
        sapool = ctx.enter_context(tc.tile_pool(name="sapool", bufs=3))
        bpool = ctx.enter_context(tc.tile_pool(name="bpool", bufs=3))
        hrpool = ctx.enter_context(tc.tile_pool(name="hrpool", bufs=2))
        hipool = ctx.enter_context(tc.tile_pool(name="hipool", bufs=2))
        tmppool = ctx.enter_context(tc.tile_pool(name="tmppool", bufs=2))
        ypool = ctx.enter_context(tc.tile_pool(name="ypool", bufs=3))
        opool = ctx.enter_context(tc.tile_pool(name="opool", bufs=2))
        touchpool = ctx.enter_context(tc.tile_pool(name="touchpool", bufs=2))
        psN = ctx.enter_context(tc.tile_pool(name="psN", bufs=2, space="PSUM"))
        psA = ctx.enter_context(tc.tile_pool(name="psA", bufs=2, space="PSUM"))
        psG = ctx.enter_context(tc.tile_pool(name="psG", bufs=2, space="PSUM"))
        psO = ctx.enter_context(tc.tile_pool(name="psO", bufs=2, space="PSUM"))

        sb_win = singles.tile([128, 3, N_PASS, 2 * C], BF16)
        nc.sync.dma_start(out=sb_win[:], in_=w_in[:])
        if out_fp8:
            sb_wout = singles.tile([C, 3, N_PAIR, 2, C], FP8)
        else:
            sb_wout = singles.tile([C, 27, C], BF16)
        nc.sync.dma_start(out=sb_wout[:], in_=w_out[:])
        sb_ones = singles.tile([C, 128], BF16)
        nc.sync.dma_start(out=sb_ones[:], in_=onesw[:])
        sb_c = singles.tile([C, 14], F32)
        nc.sync.dma_start(out=sb_c[:], in_=consts[:])
        c_ba = sb_c[:, 0:1]
        c_bg = sb_c[:, 1:2]
        c_lr = sb_c[:, 2:3]
        c_li = sb_c[:, 3:4]
        c_nli = sb_c[:, 4:5]
        c_gcre = sb_c[:, 5:6]
        c_gcim = sb_c[:, 6:7]
        c_bout = sb_c[:, 7:8]
        c_m0 = sb_c[:, 8:9]
        c_1m0 = sb_c[:, 9:10]
        c_m1 = sb_c[:, 10:11]
        c_1m1 = sb_c[:, 11:12]
        c_eps = sb_c[:, 12:13]
        c_wsc = sb_c[:, 13:14]  # 1/W_SCALE (fp8) or 1.0

        # Warm-up reads: make each compute engine observe the const-DMA
        # semaphores early, so steady-state ops carry at most one sync wait
        # (walrus rejects DVE ops with two wait commands).
        wu_v = singles.tile([C, 14], F32)
        nc.vector.tensor_copy(wu_v[:], sb_c[:])
        wu_s = singles.tile([C, 14], F32)
        nc.scalar.activation(wu_s[:], sb_c[:], AF.Square)

        def touch(ap, engines="v"):
            """Tiny read of a freshly-DMA'd tile so the engine observes the
            DMA-queue semaphore here; later big consumers then carry only
            engine-sem waits (walrus rejects DVE ops with 2 sync waits)."""
            if "v" in engines:
                tv = touchpool.tile([C, 1], F32, tag="tv")
                nc.vector.tensor_copy(tv[:], ap)
            if "s" in engines:
                ts_ = touchpool.tile([C, 1], F32, tag="ts")
                nc.scalar.activation(ts_[:], ap, AF.Square)

        hn_slabs = [None] * T  # hnorm tiles [128, N_CFG, NPL], data at col 1
        y_slabs = [None] * T   # y tiles [C, NPLU], data at col 1

        def stage_a(t):
            """x[t] -> hnorm[t] (rms-normed, bf16, [128, N_CFG, NPL])."""
            xt = xpool.tile([C, RIN, Wp], F32, tag="xt")
            nc.sync.dma_start(out=xt[:], in_=xh[:, t])
            touch(xt[:, 0, 0:1], engines="vs")
            xf = xt[:].rearrange("p r w -> p (r w)")
            hn = hnpool.tile([128, N_CFG, NPL], BF16, tag="hn")
            for p in range(N_CFG):
                nc.vector.memset(hn[0:C, p, 0:1], 0.0)
                nc.vector.memset(hn[0:C, p, 1 + NIN:], 0.0)
            for c0, n in col_tiles(NIN):
                sq = sqpool.tile([C, CT], BF16, tag="sq")
                nc.scalar.activation(sq[:, :n], xf[:, c0:c0 + n], AF.Square)
                ps = psN.tile([128, CT], F32, tag="psn")
                nc.tensor.matmul(ps[:, :n], sb_ones[:], sq[:, :n],
                                 start=True, stop=True)
                lg = statpool.tile([C, CT], F32, tag="lg")
                nc.scalar.activation(lg[:, :n], ps[:C, :n], AF.Ln,
                                     scale=1.0 / C, bias=c_eps)
                inv = statpool.tile([C, CT], F32, tag="inv")
                nc.scalar.activation(inv[:, :n], lg[:, :n], AF.Exp, scale=-0.5)
                for p in range(N_CFG):
                    nc.vector.tensor_mul(hn[0:C, p, 1 + c0:1 + c0 + n],
                                         xf[:, c0:c0 + n], inv[:, :n])
            # partitions 96:128 of each plane: pre-shifted 32-ch group copy
            for c, (g, rho) in enumerate(CFGS):
                nc.sync.dma_start(
                    out=hn[96:128, c, 0:NPL - rho],
                    in_=hn[g * 32:(g + 1) * 32, 0, rho:NPL])
                nc.vector.memset(hn[96:128, c, NPL - rho:NPL], 0.0)
            hn_slabs[t] = hn
            return hn

        def gate_epilogue(pa, pg, bt, c0, n):
            """silu(a+ba)*(g+bg) for one coltile; a=pa[0:C], g=pg[0:C]."""
            if use_silu:
                sa = sapool.tile([C, CT], BF16, tag="sa")
                nc.scalar.activation(sa[:, :n], pa[:C, :n], AF.Silu, bias=c_ba)
            else:
                sg = sapool.tile([C, CT], BF16, tag="sg")
                nc.scalar.activation(sg[:, :n], pa[:C, :n], AF.Sigmoid,
                                     bias=c_ba)
                av = sapool.tile([C, CT], F32, tag="av")
                nc.vector.scalar_tensor_tensor(av[:, :n], pa[:C, :n], c_ba,
                                               sg[:, :n], ALU.add, ALU.bypass)
                sa = sapool.tile([C, CT], BF16, tag="sa")
                nc.vector.tensor_mul(sa[:, :n], sg[:, :n], av[:, :n])
            nc.vector.scalar_tensor_tensor(bt[:, c0:c0 + n], pg[:C, :n],
                                           c_bg, sa[:, :n],
                                           ALU.add, ALU.mult)

        scan_state = [None, None]  # hr, hi tiles [C, NU] bf16

        def conv_in_scan(t):
            """hnorm[t-1..t+1] -> u[t] -> LRU step -> y[t] bulk (per-coltile
            chunked so the tensor stream is never far ahead of y)."""
            slabs = [hn_slabs[min(max(t + kt - 1, 0), T - 1)] for kt in range(3)]
            hr_old, hi_old = scan_state
            hr_new = hrpool.tile([C, NU], BF16, tag="hr")
            hi_new = hipool.tile([C, NU], BF16, tag="hi")
            yt = ypool.tile([C, NPLU], YDT, tag="yt")
            nc.vector.memset(yt[:, 0:1], 0.0)
            nc.vector.memset(yt[:, 1 + NU:], 0.0)
            n_mm = 3 * N_PASS
            for c0, n in col_tiles(NU):
                pa = psA.tile([C, CT], F32, tag="pa")
                pg = psG.tile([C, CT], F32, tag="pg")
                for half, ps in ((0, pa), (1, pg)):
                    idx = 0
                    for kt in range(3):
                        slab = slabs[kt]
                        for p, ((kh, kw), cfg, _tapb) in enumerate(PASSES):
                            s = c0 + kh * Wp + kw
                            rhs = slab[:, cfg, s:s + n]
                            nc.tensor.matmul(
                                ps[:, :n],
                                sb_win[:, kt, p, half * C:(half + 1) * C],
                                rhs, start=(idx == 0), stop=(idx == n_mm - 1))
                            idx += 1
                bt = bpool.tile([C, CT], BF16, tag="bt")
                gate_epilogue(pa, pg, bt, 0, n)
                # LRU chunk: state update + projection for cols [c0, c0+n)
                hrs = hr_new[:, c0:c0 + n]
                his = hi_new[:, c0:c0 + n]
                if t == 0:
                    nc.vector.tensor_copy(hrs, bt[:, :n])
                    nc.vector.memset(his, 0.0)
                else:
                    t1 = tmppool.tile([C, CT], BF16, tag="tA")
                    nc.vector.scalar_tensor_tensor(
                        t1[:, :n], hi_old[:, c0:c0 + n], c_nli, bt[:, :n],
                        ALU.mult, ALU.add)
                    nc.vector.scalar_tensor_tensor(
                        hrs, hr_old[:, c0:c0 + n], c_lr, t1[:, :n],
                        ALU.mult, ALU.add)
                    t2 = tmppool.tile([C, CT], BF16, tag="tB")
                    nc.vector.scalar_tensor_tensor(
                        t2[:, :n], hi_old[:, c0:c0 + n], c_lr,
                        hi_old[:, c0:c0 + n], ALU.mult, ALU.bypass)
                    nc.vector.scalar_tensor_tensor(
                        his, hr_old[:, c0:c0 + n], c_li, t2[:, :n],
                        ALU.mult, ALU.add)
                t3 = tmppool.tile([C, CT], BF16, tag="tC")
                nc.vector.scalar_tensor_tensor(t3[:, :n], hrs, c_gcre, hrs,
                                               ALU.mult, ALU.bypass)
                nc.vector.scalar_tensor_tensor(yt[:, 1 + c0:1 + c0 + n], his,
                                               c_gcim, t3[:, :n],
                                               ALU.mult, ALU.add)
            scan_state[0], scan_state[1] = hr_new, hi_new
            # W wrap columns: col 0 <- col W (w=W-1), col W+1 <- col 1 (w=0)
            yv = yt[:, 1:1 + NU].rearrange("p (r w) -> p r w", w=Wp)
            nc.vector.tensor_copy(yv[:, :, 0:1], yv[:, :, W:W + 1])
            nc.vector.tensor_copy(yv[:, :, W + 1:W + 2], yv[:, :, 1:2])
            # H edge replication (active only on global-edge cores, via mask):
            # row0 <- m0*row0 + (1-m0)*row1 ; last <- m1*last + (1-m1)*prev
            e0 = tmppool.tile([C, Wp], F32, tag="tE")
            nc.vector.scalar_tensor_tensor(e0[:], yv[:, 1, :], c_1m0,
                                           yv[:, 1, :], ALU.mult, ALU.bypass)
            nc.vector.scalar_tensor_tensor(yv[:, 0, :], yv[:, 0, :], c_m0,
                                           e0[:], ALU.mult, ALU.add)
            e1 = tmppool.tile([C, Wp], F32, tag="tE")
            nc.vector.scalar_tensor_tensor(e1[:], yv[:, RU - 2, :], c_1m1,
                                           yv[:, RU - 2, :], ALU.mult, ALU.bypass)
            nc.vector.scalar_tensor_tensor(yv[:, RU - 1, :], yv[:, RU - 1, :],
                                           c_m1, e1[:], ALU.mult, ALU.add)
            y_slabs[t] = yt
            return yt

        def conv_out(t):
            """y[t-1..t+1] -> out[t] = x + conv(y) + b_out."""
            slabs = [y_slabs[min(max(t + kt - 1, 0), T - 1)] for kt in range(3)]
            ot = opool.tile([C, HR, Wp], F32, tag="ot")
            # residual input loaded into the output staging tile
            nc.sync.dma_start(out=ot[:], in_=xh[:, t, 2:2 + HR, :])
            touch(ot[:, 0, 0:1], engines="v")
            of = ot[:].rearrange("p r w -> p (r w)")
            if out_fp8:
                # pre-add b_out to the residual staging tile
                nc.vector.scalar_tensor_tensor(of[:, :], of[:, :], c_bout,
                                               of[:, :], ALU.add, ALU.bypass)
                n_mm = 3 * N_PAIR
                for c0, n in col_tiles(NO):
                    po = psO.tile([C, CT], F32, tag="po")
                    idx = 0
                    for kt in range(3):
                        slab = slabs[kt]
                        for p, (tap1, tap2) in enumerate(OUT_PAIRS):
                            d1 = tap1[0] * Wp + tap1[1]
                            d2 = (tap2[0] * Wp + tap2[1]) if tap2 else d1
                            rhs = slab[:, c0 + d1:c0 + d1 + n].unsqueeze(1)
                            rhs.ap[1] = (d2 - d1, 2)
                            rhs.ap[2] = (1, n)
                            nc.tensor.matmul(
                                po[:, :n], sb_wout[:, kt, p], rhs,
                                start=(idx == 0), stop=(idx == n_mm - 1),
                                perf_mode=mybir.MatmulPerfMode.DoubleRow)
                            idx += 1
                    nc.vector.scalar_tensor_tensor(of[:, c0:c0 + n],
                                                   po[:, :n], c_wsc,
                                                   of[:, c0:c0 + n],
                                                   ALU.mult, ALU.add)
            else:
                n_mm = 27
                for c0, n in col_tiles(NO):
                    po = psO.tile([C, CT], F32, tag="po")
                    idx = 0
                    for kt in range(3):
                        slab = slabs[kt]
                        for kh in range(3):
                            for kw in range(3):
                                s = c0 + kh * Wp + kw
                                nc.tensor.matmul(
                                    po[:, :n], sb_wout[:, idx % 27, :],
                                    slab[:, s:s + n],
                                    start=(idx == 0), stop=(idx == n_mm - 1))
                                idx += 1
                    nc.vector.scalar_tensor_tensor(of[:, c0:c0 + n],
                                                   po[:, :n], c_bout,
                                                   of[:, c0:c0 + n],
                                                   ALU.add, ALU.add)
            nc.sync.dma_start(out=out[:, t], in_=ot[:, :, 1:1 + W])

        for t in range(min(2, T)):
            stage_a(t)
        for t in range(T):
            if t + 2 < T:
                stage_a(t + 2)
            conv_in_scan(t)
            if t >= 1:
                conv_out(t - 1)
        conv_out(T - 1)

    nc.compile()
    return nc


def prep_weight_aug(w_t, Wp, n_out):
    """Pack [C, 27, n_out] tap-major weights into the augmented layout
    [128, 3, N_PASS, n_out]: rows 0:96 = base tap A, rows 96:128 = the
    covered leftover unit's 32-channel group (or zero)."""
    C = w_t.shape[0]
    CFGS, PASSES = make_schedule(Wp)
    w_aug = np.zeros((128, 3, N_PASS, n_out), np.float32)
    for kt in range(3):
        for p, ((kh, kw), cfg, tapb) in enumerate(PASSES):
            off = kt * 9 + kh * 3 + kw
            w_aug[0:C, kt, p] = w_t[:, off]
            if tapb is not None:
                g, _rho = CFGS[cfg]
                off_b = kt * 9 + tapb[0] * 3 + tapb[1]
                w_aug[96:128, kt, p] = w_t[g * 32:(g + 1) * 32, off_b]
    return w_aug.astype(ml_dtypes.bfloat16)


def prep_weight_pairs_fp8(w_t, n_out):
    """Pack [C, 27, n_out] tap-major weights into DoubleRow pair layout
    [C, 3, N_PAIR, 2, n_out] fp8, scaled by W_SCALE."""
    C = w_t.shape[0]
    w_p = np.zeros((C, 3, N_PAIR, 2, n_out), np.float32)
    for kt in range(3):
        for p, (tap1, tap2) in enumerate(OUT_PAIRS):
            w_p[:, kt, p, 0] = w_t[:, kt * 9 + tap1[0] * 3 + tap1[1]]
            if tap2 is not None:
                w_p[:, kt, p, 1] = w_t[:, kt * 9 + tap2[0] * 3 + tap2[1]]
    return np.clip(w_p * W_SCALE, -240.0, 240.0).astype(E4NP)


def prep_core_inputs(x, norm_w, conv_in_w, conv_in_b, nu_log, theta_log,
                     c_re, c_im, conv_out_w, conv_out_b, n_qh, out_fp8=True):
    """Build per-core input maps. Cores = batch-major, then H quarters."""
    B, C, T, H, W = x.shape
    HR = H // n_qh
    Wp = W + 2

    nu = np.exp(np.asarray(nu_log, np.float64))
    theta = np.exp(np.asarray(theta_log, np.float64))
    lam_re = (np.exp(-nu) * np.cos(theta)).astype(np.float32)
    lam_im = (np.exp(-nu) * np.sin(theta)).astype(np.float32)
    gamma = np.sqrt(1.0 - np.exp(-2.0 * nu))
    gcre = (gamma * np.asarray(c_re, np.float64)).astype(np.float32)
    gcim = (gamma * np.asarray(c_im, np.float64)).astype(np.float32)

    w_in_f = np.asarray(conv_in_w, np.float32) * \
        np.asarray(norm_w, np.float32)[None, :, None, None, None]
    w_in_t = np.ascontiguousarray(
        np.transpose(w_in_f, (1, 2, 3, 4, 0)).reshape(C, 27, 2 * C))
    w_out_t = np.ascontiguousarray(
        np.transpose(np.asarray(conv_out_w, np.float32),
                     (1, 2, 3, 4, 0)).reshape(C, 27, C))
    w_in_aug = prep_weight_aug(w_in_t, Wp, 2 * C)
    if out_fp8:
        w_out_k = prep_weight_pairs_fp8(w_out_t, C)
        wsc = np.full(C, 1.0 / W_SCALE, np.float32)
    else:
        w_out_k = w_out_t.astype(ml_dtypes.bfloat16)
        wsc = np.ones(C, np.float32)
    ones = np.ones((C, 128), ml_dtypes.bfloat16)

    xp = np.concatenate([x[..., -1:], x, x[..., :1]], axis=-1)  # W circular

    in_maps = []
    for b in range(B):
        for q in range(n_qh):
            rows = np.clip(np.arange(q * HR - 2, q * HR + HR + 2), 0, H - 1)
            xh = np.ascontiguousarray(xp[b][:, :, rows, :]).astype(np.float32)
            m0 = 0.0 if q == 0 else 1.0
            m1 = 0.0 if q == n_qh - 1 else 1.0
            cvec = np.stack([
                np.asarray(conv_in_b, np.float32)[:C],
                np.asarray(conv_in_b, np.float32)[C:],
                lam_re, lam_im, -lam_im, gcre, gcim,
                np.asarray(conv_out_b, np.float32),
                np.full(C, m0, np.float32), np.full(C, 1.0 - m0, np.float32),
                np.full(C, m1, np.float32), np.full(C, 1.0 - m1, np.float32),
                np.full(C, EPS, np.float32),
                wsc,
            ], axis=1)
            in_maps.append({
                "xh": xh,
                "w_in": w_in_aug,
                "w_out": w_out_k,
                "onesw": ones,
                "consts": np.ascontiguousarray(cvec),
            })
    return in_maps


LAST_RESULT = None  # BassKernelResults of the most recent kernel() call


def kernel(x, norm_w, conv_in_w, conv_in_b, nu_log, theta_log, c_re, c_im,
           conv_out_w, conv_out_b):
    global LAST_RESULT
    from concourse.bass_utils import run_bass_kernel_spmd

    x = np.asarray(x, np.float32)
    B, C, T, H, W = x.shape
    HR = H // QH
    out_fp8 = os.environ.get("KERNEL_OUT_FP8", "1") == "1"
    in_maps = prep_core_inputs(x, norm_w, conv_in_w, conv_in_b, nu_log,
                               theta_log, c_re, c_im, conv_out_w, conv_out_b,
                               QH, out_fp8=out_fp8)
    nc = build_program(C=C, T=T, HR=HR, W=W, CT=512, out_fp8=out_fp8)
    trace = os.environ.get("KERNEL_TRACE", "") == "1"
    res = run_bass_kernel_spmd(nc, in_maps, list(range(N_CORES)), trace=trace)
    LAST_RESULT = res
    out = np.empty((B, C, T, H, W), np.float32)
    for core in range(N_CORES):
        b, q = core // QH, core % QH
        out[b, :, :, q * HR:(q + 1) * HR, :] = res.results[core]["out"]
    return out
